# revision 1
# baseline (speedup 1.0000x reference)
"""Exphormer edge-attention kernel for 8 Trainium2 NeuronCores.

Strategy:
  - Host: bucket edges by 128-node destination window (graph partition).
    Core c owns 49 windows (6272 nodes). Every window is padded to a
    uniform number of 128-edge blocks so one SPMD program fits all cores.
  - Device (per core):
      prologue: build Q/K/V projection tables (bf16) for ALL nodes via
        PE matmuls from a channel-major copy of h.
      main loop: for each window, stream edge blocks:
        * load edge_attr^T block (f32), cast bf16 (ACT)
        * indirect-gather K|V rows (packed, bf16) by src, Q rows by dst
        * Eh = ea @ WE on PE (psum f32), cast bf16
        * t = (K*Q)*Eh on DVE, per-head segmented reduce -> raw score
        * clip +-20 (DVE), exp(0.25*x) on ACT -> score
        * msg = V * score (DVE); rhs = [msg | score]
        * scatter: psum[128 nodes,136] += onehot(dst)^T @ rhs on PE,
          accumulated across the whole window in one PSUM bank
      window epilogue: out = wV / (Z + 1e-6), DMA to output slice.
  - Host: concatenate core outputs, trim padding rows.
"""

import math
import os
import sys
from contextlib import ExitStack

import numpy as np

for _p in ("/opt/trn_rl_repo", "/root/.axon_site/_ro/trn_rl_repo"):
    if os.path.isdir(_p) and _p not in sys.path:
        sys.path.insert(0, _p)

import ml_dtypes  # noqa: E402

import concourse.bass as bass  # noqa: E402
import concourse.tile as tile  # noqa: E402
from concourse import bacc, mybir  # noqa: E402
from concourse.bass import IndirectOffsetOnAxis  # noqa: E402
from concourse.bass_utils import run_bass_kernel_spmd  # noqa: E402

F32 = mybir.dt.float32
BF16 = mybir.dt.bfloat16
I32 = mybir.dt.int32
ALU = mybir.AluOpType
ACTF = mybir.ActivationFunctionType

N_NODES = 50000
N_EDGES = 1600000
DIM = 128
H = 8
D = 16
N_CORES = 8
P = 128  # partitions / window size / edge-block size
QUAD = 4  # edge blocks fused per pipeline step

LAST_EXEC_NS = None
LAST_NC = None
LAST_IN_MAPS = None


def build_program(n_cores, wpc, bpg, ablate=frozenset(), repeat=1):
    """Build + compile the SPMD Tile program.

    wpc: windows per core; bpg: 128-edge blocks per window (uniform).
    ablate/repeat are for benchmarking only.
    """
    nb = wpc * bpg  # blocks per core
    n_pad = n_cores * wpc * P  # padded node count (table rows)

    nc = bacc.Bacc(
        "TRN2", target_bir_lowering=False, debug=False, num_devices=n_cores,
        num_swdge_queues=4,
    )

    ea_t = nc.dram_tensor("eaT", [P, nb * P], F32, kind="ExternalInput").ap()
    src_t = nc.dram_tensor("srcT", [P, nb], I32, kind="ExternalInput").ap()
    dst_t = nc.dram_tensor("dstT", [P, nb], I32, kind="ExternalInput").ap()
    dstrel_t = nc.dram_tensor("dstrelT", [P, nb], F32, kind="ExternalInput").ap()
    h_t = nc.dram_tensor("hT", [P, n_pad], F32, kind="ExternalInput").ap()
    w3 = nc.dram_tensor("w3", [P, 3 * DIM], F32, kind="ExternalInput").ap()
    we = nc.dram_tensor("we", [P, DIM], F32, kind="ExternalInput").ap()
    iota = nc.dram_tensor("iota4", [P, QUAD * P], F32, kind="ExternalInput").ap()
    out = nc.dram_tensor("out", [wpc * P, DIM], F32, kind="ExternalOutput").ap()

    q_tab = nc.dram_tensor("q_tab", [n_pad, DIM], BF16).ap()
    kv_tab = nc.dram_tensor("kv_tab", [n_pad, 2 * DIM], BF16).ap()

    with tile.TileContext(nc) as tc, ExitStack() as ctx:
        singles = ctx.enter_context(tc.tile_pool(name="singles", bufs=1))
        tbl = ctx.enter_context(tc.tile_pool(name="tbl", bufs=3))
        tbl_ps = ctx.enter_context(tc.tile_pool(name="tbl_ps", bufs=2, space="PSUM"))
        stream = ctx.enter_context(tc.tile_pool(name="stream", bufs=5))
        eh_ps = ctx.enter_context(tc.tile_pool(name="eh_ps", bufs=3, space="PSUM"))
        acc_ps = ctx.enter_context(tc.tile_pool(name="acc_ps", bufs=2, space="PSUM"))
        evac = ctx.enter_context(tc.tile_pool(name="evac", bufs=3))

        # ---- constants / resident index tables ----
        w3f = singles.tile([P, 3 * DIM], F32)
        nc.sync.dma_start(out=w3f[:], in_=w3[:])
        w3b = singles.tile([P, 3 * DIM], BF16)
        nc.scalar.activation(out=w3b[:], in_=w3f[:], func=ACTF.Copy)
        wef = singles.tile([P, DIM], F32)
        nc.sync.dma_start(out=wef[:], in_=we[:])
        web = singles.tile([P, DIM], BF16)
        nc.scalar.activation(out=web[:], in_=wef[:], func=ACTF.Copy)
        iot = singles.tile([P, QUAD, P], F32)
        nc.sync.dma_start(out=iot[:], in_=iota[:].rearrange("p (q n) -> p q n", q=QUAD))
        src_all = singles.tile([P, nb], I32)
        nc.sync.dma_start(out=src_all[:], in_=src_t[:])
        dst_all = singles.tile([P, nb], I32)
        nc.sync.dma_start(out=dst_all[:], in_=dst_t[:])
        drel_all = singles.tile([P, nb], F32)
        nc.sync.dma_start(out=drel_all[:], in_=dstrel_t[:])

        def emit_rep():
            # ---- prologue: projection tables for all (padded) nodes ----
            nquads = (bpg + QUAD - 1) // QUAD
            if "notables" not in ablate:
                for t in range(n_pad // P):
                    hf = tbl.tile([P, P], F32)
                    nc.sync.dma_start(out=hf[:], in_=h_t[:, t * P : (t + 1) * P])
                    hb = tbl.tile([P, P], BF16)
                    nc.scalar.activation(out=hb[:], in_=hf[:], func=ACTF.Copy)
                    qkv = tbl_ps.tile([P, 3 * DIM], F32, space="PSUM")
                    nc.tensor.matmul(
                        out=qkv[:], lhsT=hb[:], rhs=w3b[:], start=True, stop=True
                    )
                    qt = tbl.tile([P, DIM], BF16)
                    nc.scalar.activation(out=qt[:], in_=qkv[:, 0:DIM], func=ACTF.Copy)
                    kvt = tbl.tile([P, 2 * DIM], BF16)
                    nc.vector.tensor_copy(out=kvt[:], in_=qkv[:, DIM : 3 * DIM])
                    nc.sync.dma_start(out=q_tab[t * P : (t + 1) * P, :], in_=qt[:])
                    nc.sync.dma_start(out=kv_tab[t * P : (t + 1) * P, :], in_=kvt[:])

            # ---- main loop ----
            for w in range(wpc):
                acc = acc_ps.tile([P, DIM + H], F32, space="PSUM")
                for qd in range(nquads):
                    k0 = qd * QUAD
                    kn = min(QUAD, bpg - k0)
                    b0 = w * bpg + k0  # first block of this quad
                    fe = kn * P  # free-dim elements (edges in quad)

                    eab = stream.tile([P, fe], BF16, tag="eab")
                    if "noea" not in ablate:
                        eaf = stream.tile([P, fe], F32, tag="eaf")
                        nc.sync.dma_start(
                            out=eaf[:], in_=ea_t[:, b0 * P : b0 * P + fe]
                        )
                        nc.scalar.activation(out=eab[:], in_=eaf[:], func=ACTF.Copy)

                    kv4 = stream.tile([P, kn, 2 * DIM], BF16, tag="kv4")
                    q4 = stream.tile([P, kn, DIM], BF16, tag="q4")
                    for k in range(kn):
                        seq = ((b0 + k) % (n_pad // P - 1)) * P
                        if "seqgather" in ablate:
                            nc.sync.dma_start(
                                out=kv4[:, k, :], in_=kv_tab[seq : seq + P, :]
                            )
                        else:
                            qn = (2 * (b0 + k)) % 4
                            ins = nc.gpsimd.indirect_dma_start(
                                out=kv4[:, k, :],
                                out_offset=None,
                                in_=kv_tab[:],
                                in_offset=IndirectOffsetOnAxis(
                                    ap=src_all[:, b0 + k : b0 + k + 1], axis=0
                                ),
                            )
                            ins.ins.queue = f"qPoolDynamic{qn or ''}" 
                        if "noq" in ablate:
                            continue
                        if "seqgather" in ablate:
                            nc.sync.dma_start(
                                out=q4[:, k, :], in_=q_tab[seq : seq + P, :]
                            )
                        else:
                            qn = (2 * (b0 + k) + 1) % 4
                            ins = nc.gpsimd.indirect_dma_start(
                                out=q4[:, k, :],
                                out_offset=None,
                                in_=q_tab[:],
                                in_offset=IndirectOffsetOnAxis(
                                    ap=dst_all[:, b0 + k : b0 + k + 1], axis=0
                                ),
                            )
                            ins.ins.queue = f"qPoolDynamic{qn or ''}" 

                    ehp = eh_ps.tile([P, QUAD * P], F32, space="PSUM", tag="ehp")
                    for k in range(kn):
                        nc.tensor.matmul(
                            out=ehp[:, k * P : (k + 1) * P],
                            lhsT=eab[:, k * P : (k + 1) * P],
                            rhs=web[:],
                            start=True,
                            stop=True,
                        )
                    ehs = stream.tile([P, fe], BF16, tag="ehs")
                    nc.scalar.activation(out=ehs[:], in_=ehp[:, 0:fe], func=ACTF.Copy)

                    kq = stream.tile([P, fe], BF16, tag="kq")
                    nc.vector.tensor_tensor(
                        out=kq[:].rearrange("p (q n) -> p q n", q=kn),
                        in0=kv4[:, :, 0:DIM],
                        in1=kv4[:, :, 0:DIM] if "noq" in ablate else q4[:],
                        op=ALU.mult,
                    )
                    t3 = stream.tile([P, fe], BF16, tag="t3")
                    nc.vector.tensor_tensor(
                        out=t3[:], in0=kq[:], in1=ehs[:], op=ALU.mult
                    )
                    sraw = stream.tile([P, kn, H], F32, tag="sraw")
                    nc.vector.tensor_reduce(
                        out=sraw[:],
                        in_=t3[:].rearrange("p (q h d) -> p q h d", q=kn, h=H),
                        axis=mybir.AxisListType.X,
                        op=ALU.add,
                    )
                    sclip = stream.tile([P, kn, H], F32, tag="sclip")
                    nc.vector.tensor_scalar(
                        out=sclip[:],
                        in0=sraw[:],
                        scalar1=20.0,
                        scalar2=-20.0,
                        op0=ALU.min,
                        op1=ALU.max,
                    )
                    rhs4 = stream.tile([P, kn, DIM + H], BF16, tag="rhs4")
                    nc.scalar.activation(
                        out=rhs4[:, :, DIM : DIM + H],
                        in_=sclip[:],
                        func=ACTF.Exp,
                        scale=0.25,
                    )
                    nc.vector.tensor_tensor(
                        out=rhs4[:, :, 0:DIM].rearrange("p q (h d) -> p q h d", h=H),
                        in0=kv4[:, :, DIM : 2 * DIM].rearrange(
                            "p q (h d) -> p q h d", h=H
                        ),
                        in1=rhs4[:, :, DIM : DIM + H]
                        .unsqueeze(-1)
                        .to_broadcast((P, kn, H, D)),
                        op=ALU.mult,
                    )
                    oh = stream.tile([P, kn, P], BF16, tag="oh")
                    nc.vector.tensor_tensor(
                        out=oh[:],
                        in0=drel_all[:, b0 : b0 + kn]
                        .unsqueeze(-1)
                        .to_broadcast((P, kn, P)),
                        in1=iot[:, 0:kn, :],
                        op=ALU.is_equal,
                    )
                    for k in range(kn):
                        nc.tensor.matmul(
                            out=acc[:],
                            lhsT=oh[:, k, :],
                            rhs=rhs4[:, k, :],
                            start=(qd == 0 and k == 0),
                            stop=(qd == nquads - 1 and k == kn - 1),
                        )

                # ---- window epilogue: normalize + store ----
                zeps = evac.tile([P, H], F32, tag="zeps")
                nc.vector.tensor_scalar_add(zeps[:], acc[:, DIM : DIM + H], 1e-6)
                rec = evac.tile([P, H], F32, tag="rec")
                nc.vector.reciprocal(out=rec[:], in_=zeps[:])
                hout = evac.tile([P, DIM], F32, tag="hout")
                nc.vector.tensor_tensor(
                    out=hout[:].rearrange("p (h d) -> p h d", h=H),
                    in0=acc[:, 0:DIM].rearrange("p (h d) -> p h d", h=H),
                    in1=rec[:].unsqueeze(-1).to_broadcast((P, H, D)),
                    op=ALU.mult,
                )
                nc.sync.dma_start(out=out[w * P : (w + 1) * P, :], in_=hout[:])

        for _rep in range(repeat):
            emit_rep()

    nc.compile()
    return nc


def shard_inputs(h, edge_attr, WQ, WK, WV, WE, edge_index, n_cores, n_nodes):
    """Bucket edges by destination window; build per-core device arrays."""
    src = np.asarray(edge_index[0]).astype(np.int64)
    dst = np.asarray(edge_index[1]).astype(np.int64)
    e = src.shape[0]
    nwin = (n_nodes + P - 1) // P
    wpc = (nwin + n_cores - 1) // n_cores
    nwin_pad = n_cores * wpc
    n_pad = nwin_pad * P

    win = dst >> 7
    counts = np.bincount(win, minlength=nwin_pad)
    bpg = max(1, int(math.ceil(counts.max() / P)))
    slots_per_win = bpg * P
    nslots = nwin_pad * slots_per_win

    order = np.argsort(win, kind="stable")
    win_sorted = win[order]
    cumstarts = np.concatenate(([0], np.cumsum(counts)[:-1]))
    within = np.arange(e, dtype=np.int64) - cumstarts[win_sorted]
    slot_of_edge = win_sorted * slots_per_win + within

    slot_eid = np.full(nslots, -1, dtype=np.int64)
    slot_eid[slot_of_edge] = order

    mask = slot_eid >= 0
    eid_safe = np.where(mask, slot_eid, 0)
    src_slot = np.where(mask, src[eid_safe], 0).astype(np.int32)
    win_base = (
        (np.arange(nslots, dtype=np.int64) // slots_per_win) * P
    )
    dst_slot = np.where(mask, dst[eid_safe], win_base).astype(np.int32)
    drel_slot = np.where(mask, dst[eid_safe] - win_base, -1).astype(np.float32)

    ea = np.asarray(edge_attr, dtype=np.float32)
    h = np.asarray(h, dtype=np.float32)

    h_pad_t = np.zeros((DIM, n_pad), dtype=np.float32)
    h_pad_t[:, :n_nodes] = h.T
    w3 = np.ascontiguousarray(
        np.concatenate(
            [np.asarray(WQ), np.asarray(WK), np.asarray(WV)], axis=1
        ).astype(np.float32)
    )
    we = np.ascontiguousarray(np.asarray(WE, dtype=np.float32))
    iota4 = np.tile(np.arange(P, dtype=np.float32), (P, QUAD))
    iota4 = np.ascontiguousarray(iota4)

    nb = wpc * bpg  # blocks per core
    spc = nb * P  # slots per core
    in_maps = []
    for c in range(n_cores):
        sl = slice(c * spc, (c + 1) * spc)
        eid_c = slot_eid[sl]
        m_c = eid_c >= 0
        ea_c = np.zeros((spc, DIM), dtype=np.float32)
        ea_c[m_c] = ea[eid_c[m_c]]
        in_maps.append(
            {
                "eaT": np.ascontiguousarray(ea_c.T),
                "srcT": np.ascontiguousarray(src_slot[sl].reshape(nb, P).T),
                "dstT": np.ascontiguousarray(dst_slot[sl].reshape(nb, P).T),
                "dstrelT": np.ascontiguousarray(
                    drel_slot[sl].reshape(nb, P).T
                ),
                "hT": h_pad_t,
                "w3": w3,
                "we": we,
                "iota4": iota4,
            }
        )
    return in_maps, wpc, bpg


def kernel(h, edge_attr, WQ, WK, WV, WE, edge_index):
    global LAST_EXEC_NS, LAST_NC, LAST_IN_MAPS
    n_nodes = np.asarray(h).shape[0]
    in_maps, wpc, bpg = shard_inputs(
        h, edge_attr, WQ, WK, WV, WE, edge_index, N_CORES, n_nodes
    )
    nc = build_program(N_CORES, wpc, bpg)
    LAST_NC, LAST_IN_MAPS = nc, in_maps
    res = run_bass_kernel_spmd(nc, in_maps, list(range(N_CORES)))
    LAST_EXEC_NS = res.exec_time_ns
    outs = [np.asarray(res.results[c]["out"]) for c in range(N_CORES)]
    full = np.concatenate(outs, axis=0)[:n_nodes]
    return full.astype(np.float32)

